# revision 14
# baseline (speedup 1.0000x reference)
"""Trainium2 Bass kernel: multi-table embedding gather (pooling=NONE).

Reference computation (hardcoded shapes):
    indices: [F=4, BL=204800] int   (values in [0, V))
    tables:  [F=4, V=1e6, D=64] f32
    out[f]   = tables[PERM[f]][indices[PERM[f]]]   -> [4, 204800, 64] f32
    PERM = [2, 0, 3, 1]

Strategy (model/table-parallel, per the sharding hint):
  * Fold the table permutation into global row ids g = PERM[f]*V + idx over a
    flat [4M, 64] table; shard row-wise across 8 cores (500,000 rows each).
  * Host routes every lookup to its owning core, bucketing by 32,768-row
    window so the gather uses the int16 `dma_gather` SWDGE ucode with
    1024-idx single-packet sub-gathers (64 descriptors/engine, the packet
    ceiling; multi-packet and prepare_only modes are ~10-100x slower per
    descriptor on the Q7).
  * RAW bass pipeline (no TileContext): Tile tracks each SWDGE DMA on one of
    8 DMASW semaphore lanes, which made every gather wait for the FULL
    completion of the gather 8 before it; the engines ping-ponged between
    ~4.5us of full-rate drain and ~5us of idle (measured 290us). Here each
    window's gathers share one explicit semaphore (+16/gather), so the Pool
    engine runs ~NBUF windows ahead and the SDMA engines stay saturated.
  * Within a window, granules guaranteed full on every core use an immediate
    count; the partial tail granule reads a preloaded count register.
  * Gathered f32 rows are cast to bf16 on the DVE and written back with one
    contiguous HWDGE DMA per window, alternating between the two HWDGE rings
    (sync=SP, scalar=ACT). bf16 halves write-side HBM traffic; tolerance is
    2e-2 and bf16 round-off is ~2e-3.
  * Host applies the recorded inverse permutation to scatter staged rows into
    the final [4, 204800, 64] f32 output (host-side unshard).
"""

import sys

import numpy as np

for _p in ("/opt/trn_rl_repo",):
    if _p not in sys.path:
        sys.path.insert(0, _p)

F = 4
V = 1_000_000
D = 64
BL = 204_800
PERM = (2, 0, 3, 1)

N_CORES = 8
P = 128
ROWS_TOTAL = F * BL                   # 819,200 lookups
SHARD = F * V // N_CORES              # 500,000 table rows per core
WIN = 32_768                          # int16-addressable window
N_FULL_WIN = SHARD // WIN             # 15 full windows
LAST_WIN_ROWS = SHARD - N_FULL_WIN * WIN  # 8,480
N_WIN = N_FULL_WIN + 1                # 16 windows per core

GRANULE = 1024          # idxs per dma_gather (single-packet: 64 desc/engine)
N_SWDGE_QUEUES = 4
DMA_SCRATCH = 98304     # SWDGE desc ring carveout (ring-capacity probe)
NBUF = 5                # f32 window tiles in flight (14KB/partition each)
NBUF_BF = 3             # bf16 writeback tiles (7KB/partition each)

WIN_ROWS = [WIN] * N_FULL_WIN + [LAST_WIN_ROWS]


def build_nc(pads, full_granules):
    """Per-core SPMD raw-bass program.

    pads[w]: staging capacity of window w (multiple of 128; covers the max
        distinct count across cores).
    full_granules[w]: granules of window w guaranteed full on EVERY core
        (immediate count); later granules use a runtime count register each.
    """
    import concourse.bacc as bacc
    import concourse.mybir as mybir

    cols = [p // P for p in pads]
    idx_cols = sum(p // 16 for p in pads)
    stage_rows = sum(pads)
    idx_off = np.cumsum([0] + [p // 16 for p in pads]).tolist()
    stage_off = np.cumsum([0] + list(pads)).tolist()

    # (window, granule, num_idxs, reg index or None) in issue order
    by_window = []
    n_regs = 0
    for w in range(N_WIN):
        granules = []
        ng = (pads[w] + GRANULE - 1) // GRANULE
        for s in range(ng):
            n = min(GRANULE, pads[w] - s * GRANULE)
            if s < full_granules[w]:
                granules.append((s, n, None))
            else:
                granules.append((s, n, n_regs))
                n_regs += 1
        by_window.append(granules)

    nc = bacc.Bacc(
        None,
        num_swdge_queues=N_SWDGE_QUEUES,
        dynamic_dma_scratch_size=DMA_SCRATCH,
    )
    tabs = [
        nc.declare_dram_parameter(
            f"tab{w}", [WIN_ROWS[w], D], mybir.dt.float32, isOutput=False
        )
        for w in range(N_WIN)
    ]
    idx_in = nc.declare_dram_parameter(
        "idx", [P, idx_cols], mybir.dt.int16, isOutput=False
    )
    cnt_in = nc.declare_dram_parameter(
        "cnt", [1, max(n_regs, 1)], mybir.dt.int32, isOutput=False
    )
    out = nc.declare_dram_parameter(
        "out", [stage_rows, D], mybir.dt.bfloat16, isOutput=True
    )

    regs = [
        nc.alloc_register(mybir.EngineType.Pool, f"cnt_reg{i}")
        for i in range(n_regs)
    ]
    idx_tile = nc.alloc_sbuf_tensor("idxt", [P, idx_cols], mybir.dt.int16)
    cnt_tile = nc.alloc_sbuf_tensor("cntt", [1, max(n_regs, 1)], mybir.dt.int32)
    dummy_dst = nc.alloc_sbuf_tensor("dmyd", [P, D], mybir.dt.float32)
    datas = [
        nc.alloc_sbuf_tensor(f"data{i}", [P, max(cols) * D], mybir.dt.float32)
        for i in range(NBUF)
    ]
    bfs = [
        nc.alloc_sbuf_tensor(f"bf{i}", [P, max(cols) * D], mybir.dt.bfloat16)
        for i in range(NBUF_BF)
    ]

    dsem = nc.alloc_semaphore("dsem")        # first idx slice landed
    cnt_sem = nc.alloc_semaphore("cntsem")   # cnt tensor landed
    isem = nc.alloc_semaphore("isem")        # idx window loads (16 each)
    csem = nc.alloc_semaphore("csem")        # casts completed (1 each)
    wsem_s = nc.alloc_semaphore("wsem_s")    # sync-ring writebacks (16 each)
    wsem_a = nc.alloc_semaphore("wsem_a")    # scalar-ring writebacks (16 each)
    # one DMA-completion sem per SWDGE queue (a sem is HW-locked to a single
    # queue); granules round-robin the queues and the per-window completion
    # condition is the 4 cumulative per-queue counts after that window.
    qsems = [nc.alloc_semaphore(f"qsem{q}") for q in range(N_SWDGE_QUEUES)]

    # ---- sync (SP) engine: input loads, then even-window writebacks ----
    nc.sync.dma_start(out=idx_tile[:, 0:8], in_=idx_in[:, 0:8]).then_inc(dsem, 16)
    nc.sync.dma_start(out=cnt_tile[:], in_=cnt_in[:]).then_inc(cnt_sem, 16)
    for w in range(N_WIN):
        nc.sync.dma_start(
            out=idx_tile[:, idx_off[w] : idx_off[w + 1]],
            in_=idx_in[:, idx_off[w] : idx_off[w + 1]],
        ).then_inc(isem, 16)

    # ---- Pool engine: warmup gather (pulls the ucode library reload to the
    # top of the stream, overlapping the input loads), count-register
    # preloads, then the gather stream (runs ahead of casts by NBUF windows).
    nc.gpsimd.wait_ge(dsem, 16)
    nc.gpsimd.dma_gather(
        dummy_dst[:].rearrange("p (c d) -> p c d", d=D),
        tabs[0][:],
        idx_tile[:, 0:1],
        16,
        16,
        D,
        single_packet=True,
        queue_num=0,
    ).then_inc(qsems[0], 16)
    nc.gpsimd.wait_ge(cnt_sem, 16)
    for i in range(n_regs):
        nc.gpsimd.reg_load(regs[i], cnt_tile[0:1, i : i + 1])
    qcum = [16] + [0] * (N_SWDGE_QUEUES - 1)   # warmup bumped qsems[0]
    win_qcum = []                              # per-window cumulative targets
    g_idx = 0
    for w in range(N_WIN):
        nc.gpsimd.wait_ge(isem, 16 * (w + 1))
        if w >= NBUF:
            nc.gpsimd.wait_ge(csem, w - NBUF + 1)
        data = datas[w % NBUF]
        for s, n, reg_i in by_window[w]:
            c0 = idx_off[w] + s * (GRANULE // 16)
            f0 = s * (GRANULE // P) * D
            ncols = (n + P - 1) // P
            q = g_idx % N_SWDGE_QUEUES
            nc.gpsimd.dma_gather(
                data[:, f0 : f0 + ncols * D].rearrange("p (c d) -> p c d", d=D),
                tabs[w][:],
                idx_tile[:, c0 : c0 + (n + 15) // 16],
                n,
                regs[reg_i] if reg_i is not None else n,
                D,
                single_packet=True,
                queue_num=q,
            ).then_inc(qsems[q], 16)
            qcum[q] += 16
            g_idx += 1
        win_qcum.append(list(qcum))

    # ---- DVE: per-window f32->bf16 cast once the window's gathers land ----
    prev = [0] * N_SWDGE_QUEUES
    for w in range(N_WIN):
        for q in range(N_SWDGE_QUEUES):
            if win_qcum[w][q] > prev[q]:
                nc.vector.wait_ge(qsems[q], win_qcum[w][q])
        prev = win_qcum[w]
        if w >= NBUF_BF:
            pw = w - NBUF_BF
            wsem = wsem_s if pw % 2 == 0 else wsem_a
            nc.vector.wait_ge(wsem, 16 * (pw // 2 + 1))
        nbytes = cols[w] * D
        nc.vector.tensor_scalar_mul(
            bfs[w % NBUF_BF][:, :nbytes], datas[w % NBUF][:, :nbytes], 1.0
        ).then_inc(csem, 1)

    # ---- writebacks: even windows on the sync (SP) HWDGE ring, odd on the
    # scalar (ACT) ring ----
    for w in range(N_WIN):
        eng = nc.sync if w % 2 == 0 else nc.scalar
        wsem = wsem_s if w % 2 == 0 else wsem_a
        eng.wait_ge(csem, w + 1)
        win_ap = out[stage_off[w] : stage_off[w + 1], :].rearrange(
            "(p c) d -> p (c d)", p=P
        )
        eng.dma_start(out=win_ap[:], in_=bfs[w % NBUF_BF][:, : cols[w] * D]).then_inc(
            wsem, 16
        )
    nc.sync.wait_ge(wsem_s, 16 * ((N_WIN + 1) // 2))
    nc.scalar.wait_ge(wsem_a, 16 * (N_WIN // 2))

    nc.compile()
    return nc


def route(indices):
    """Host-side routing: global ids -> per-core window buckets.

    Returns (idx_inputs [N_CORES, P, idx_cols] int16,
             dst_rows   [ROWS_TOTAL] original flat output rows, core-major,
             src_rows   [ROWS_TOTAL] staging row per lookup (same order),
             core_of    [ROWS_TOTAL] core id per lookup (same order),
             cnts       [N_CORES, n_regs] partial-granule valid counts,
             pads       [N_WIN] staging capacity per window,
             full_granules [N_WIN] granules full on every core).
    """
    idx = np.asarray(indices)
    perm = np.asarray(PERM)
    glob = (idx[perm].astype(np.int64) + (perm * V)[:, None]).reshape(-1)

    core = glob // SHARD                       # [N]
    local = glob - core * SHARD
    win = local // WIN                         # window id 0..15
    wlocal = local - win * WIN                 # 0..32767

    group = core * N_WIN + win                 # 0..127
    order = np.lexsort((wlocal, group))        # grouped; row-sorted in group
    g_sorted = group[order]
    w_sorted = wlocal[order]

    # dedup: duplicates of a row within a bucket share one gather slot
    first = np.ones(glob.size, bool)
    first[1:] = (g_sorted[1:] != g_sorted[:-1]) | (w_sorted[1:] != w_sorted[:-1])
    fc = np.cumsum(first) - 1                  # global distinct ordinal
    gstart = np.ones(glob.size, bool)
    gstart[1:] = g_sorted[1:] != g_sorted[:-1]
    gs_idx = np.flatnonzero(gstart)
    run_len = np.diff(np.append(gs_idx, glob.size))
    slot = fc - np.repeat(fc[gs_idx], run_len)  # distinct slot within bucket

    n_groups = N_CORES * N_WIN
    dist_counts = np.zeros(n_groups, np.int64)
    dist_counts[g_sorted[gs_idx]] = (
        fc[np.append(gs_idx[1:] - 1, glob.size - 1)] - fc[gs_idx] + 1
    )
    dc = dist_counts.reshape(N_CORES, N_WIN)
    # window capacity: max distinct count across cores, rounded up to 128;
    # at least one slot column so empty windows still have a home.
    pads = (np.maximum(dc.max(axis=0), 1) + P - 1) // P * P
    pads = pads.astype(np.int64)
    # granules guaranteed full on every core
    full_granules = (dc.min(axis=0) // GRANULE).astype(np.int64)

    cols = pads // P
    idx_off16 = np.cumsum([0] + (pads // 16).tolist())[:-1]
    idx_cols = int((pads // 16).sum())
    stage_off = np.cumsum([0] + pads.tolist())[:-1]

    base = stage_off[g_sorted % N_WIN]
    colw = cols[g_sorted % N_WIN]
    sub, r = slot // GRANULE, slot % GRANULE
    src_rows = base + (r % P) * colw + sub * (GRANULE // P) + r // P

    # int16 index tiles, wrapped in 16 partitions: distinct slot d goes to
    # [partition d%16, col c0_w + d//16]; unused cells are -1 (skipped by the
    # runtime count register).
    idx16 = np.full((N_CORES, 16, idx_cols), -1, dtype=np.int16)
    c_of = g_sorted // N_WIN
    flat_cols = idx_off16[g_sorted % N_WIN] + slot // 16
    idx16[c_of, slot % 16, flat_cols] = w_sorted.astype(np.int16)

    # per-register (partial-granule) valid counts; empty granules get one
    # dummy (row 0) because an all-negative gather is illegal.
    reg_specs = []
    for w in range(N_WIN):
        ng = (int(pads[w]) + GRANULE - 1) // GRANULE
        for s in range(int(full_granules[w]), ng):
            reg_specs.append((w, s))
    n_regs = len(reg_specs)
    cnts = np.zeros((N_CORES, max(n_regs, 1)), np.int32)
    for i, (w, s) in enumerate(reg_specs):
        c = np.clip(dc[:, w] - s * GRANULE, 0, GRANULE).astype(np.int32)
        empty = np.flatnonzero(c == 0)
        if empty.size:
            cells = idx_off16[w] + s * (GRANULE // 16)
            idx16[empty, 0, cells] = 0
            c[empty] = 1
        cnts[:, i] = c

    idx_inputs = np.ascontiguousarray(np.tile(idx16, (1, 8, 1)))
    return (idx_inputs, order, src_rows, c_of, cnts,
            tuple(int(x) for x in pads), tuple(int(x) for x in full_granules))


_NC_CACHE = {}


def _get_nc(pads, full_granules):
    key = (pads, full_granules)
    if key not in _NC_CACHE:
        _NC_CACHE[key] = build_nc(pads, full_granules)
    return _NC_CACHE[key]


def run_sharded(indices, tables, trace=False, **spmd_kwargs):
    """Run the SPMD kernel on 8 cores; returns (full_output, BassKernelResults)."""
    from concourse import bass_utils

    tables_flat = np.asarray(tables, dtype=np.float32).reshape(F * V, D)
    (idx_inputs, dst_rows, src_rows, core_of, cnts,
     pads, full_granules) = route(indices)

    in_maps = []
    for c in range(N_CORES):
        m = {"idx": idx_inputs[c], "cnt": cnts[c : c + 1]}
        shard = tables_flat[c * SHARD : (c + 1) * SHARD]
        r0 = 0
        for w in range(N_WIN):
            m[f"tab{w}"] = shard[r0 : r0 + WIN_ROWS[w]]
            r0 += WIN_ROWS[w]
        in_maps.append(m)

    nc = _get_nc(pads, full_granules)
    res = bass_utils.run_bass_kernel_spmd(
        nc, in_maps, list(range(N_CORES)), trace=trace, **spmd_kwargs
    )

    out_flat = np.empty((ROWS_TOTAL, D), dtype=np.float32)
    for c in range(N_CORES):
        sel = core_of == c
        staged = np.asarray(res.results[c]["out"]).astype(np.float32)
        out_flat[dst_rows[sel]] = staged[src_rows[sel]]
    return out_flat.reshape(F, BL, D), res


def kernel(indices, tables):
    out, _ = run_sharded(indices, tables, trace=False)
    return out
